# revision 4
# baseline (speedup 1.0000x reference)
"""MultiHeadAttention Trainium2 Bass kernel, v2.

Problem: B=2, S=2048, D=768, H=12 heads, head_dim=64.
    q = x@Wq+bq; k = x@Wk+bk; v = x@Wv+bv   (per-head split)
    out = softmax(q k^T / 8) v, heads merged, @ Wo + bo

Sharding (8 cores): core c handles batch b=c//4 and 3 heads (c%4)*3..+3
(Megatron attention: column-split of Wq/Wk/Wv, row-split of Wo). Each core
produces a partial [S, D] output; the host sums the 4 partials per batch and
adds (bv @ Wo + bo) once.

Timeline: 126.6us vs v1's 179.3us (1.42x). ~10.5us DMA-bound startup,
~100us ACT exp stream (the floor: 96 x [128,1024] exps at 1 col/cycle,
1.2GHz), ~12.5us tail (last unit's norm -> transpose -> out-proj -> DMA).

v2 redesign vs v1 (179.3us): the cost model charges a matmul ap_size(out)
cycles regardless of M/K, with NO overlap between matmuls. v1's ctx^T
matmuls ([65, Nq] out) wasted half the PE partition dim. v2 computes ctx in
[q=128, d] orientation (E tile as lhsT, [v|1] as rhs, N=65 per q-subtile):
ctx cost halves (98304 -> 49920 cycles) and the softmax denominator becomes
per-PARTITION (per query), so normalization is a cheap DVE tensor op instead
of the gpsimd partition-broadcast chain. ctx is then PE-transposed (4096
cycles) for the out-projection. The exp stream on ACT (96 x [128,1024] =
~100us) is the hard floor; PE (~97us) hides under it.

dtypes: moving-operand dtype sets matmul speed (f32r needs N>=256 for
1cyc/row; bf16 is 1cyc/row at any N). qT/kT stay f32r (full fp32 data);
x/v/ctx/Wv/Wo/identity are bf16 (small-N matmuls).

kernel(**inputs) takes FULL unsharded inputs and returns the FULL output.
"""

import numpy as np

import concourse.bass as bass
import concourse.mybir as mybir
import concourse.tile as tile
from concourse import bacc
from concourse.bass_utils import run_bass_kernel_spmd

F32 = mybir.dt.float32
F32R = mybir.dt.float32r
BF16 = mybir.dt.bfloat16
I16 = mybir.dt.int16

# Schraudolph exp on DVE: i16 = rint(s*SCH_C1 + SCH_C2); bitcast i16->bf16
# gives 2^n*(1+f) ~ exp(s) with max rel err ~4.3% (geometrically centered).
# DVE f32->i16 conversion is round-to-nearest (verified on hw).
SCH_C1 = 1.4426950408889634 * 128.0
SCH_C2 = 127.0 * 128.0 - 7.70

B, S, D = 2, 2048, 768
H, DH = 12, 64
NCORES = 8
HPC = 3                # heads per core
DH3 = HPC * DH         # 192 (per-core slice of the model dim)
KT = D // 128          # 6 contraction tiles for D
ST = S // 128          # 16 sequence tiles
GW = 1024              # attention q-chunk (g) width
NG = S // GW           # 2

_CACHED_NC = None


def _build_nc(debug: bool = False) -> bass.Bass:
    nc = bacc.Bacc()

    xT = nc.dram_tensor("xT", [D, S], BF16, kind="ExternalInput")
    wq = nc.dram_tensor("wq", [128, KT * 128], BF16, kind="ExternalInput")
    wk = nc.dram_tensor("wk", [128, KT * 128], BF16, kind="ExternalInput")
    wkq2 = nc.dram_tensor("wkq2", [128, KT * 128], BF16, kind="ExternalInput")
    wv = nc.dram_tensor("wv", [128, KT * DH3], BF16, kind="ExternalInput")
    wo = nc.dram_tensor("wo", [DH3, D], BF16, kind="ExternalInput")
    bias = nc.dram_tensor("bias", [128, 4], F32, kind="ExternalInput")
    ident = nc.dram_tensor("ident", [128, 128], BF16, kind="ExternalInput")
    out = nc.dram_tensor("out", [S, D], BF16, kind="ExternalOutput")
    if debug:
        d_qTA = nc.dram_tensor("d_qTA", [128, S], F32, kind="ExternalOutput")
        d_kTA = nc.dram_tensor("d_kTA", [128, S], F32, kind="ExternalOutput")
        d_qTB = nc.dram_tensor("d_qTB", [128, S], F32, kind="ExternalOutput")
        d_kTB = nc.dram_tensor("d_kTB", [128, S], F32, kind="ExternalOutput")
        d_v = nc.dram_tensor("d_v", [128, ST * HPC * (DH + 1)], F32,
                             kind="ExternalOutput")
        d_ctx = nc.dram_tensor("d_ctx", [128, ST * DH3], F32,
                               kind="ExternalOutput")
        d_ctxTA = nc.dram_tensor("d_ctxTA", [128, S], F32, kind="ExternalOutput")
        d_ctxTB = nc.dram_tensor("d_ctxTB", [64, S], F32, kind="ExternalOutput")

    with (
        tile.TileContext(nc) as tc,
        tc.tile_pool(name="big", bufs=1) as big,
        tc.tile_pool(name="work", bufs=2) as work,
        tc.tile_pool(name="expp", bufs=3) as expp,
        tc.tile_pool(name="outp", bufs=6) as outp,
        tc.tile_pool(name="psS", bufs=2, space="PSUM") as psS,
        tc.tile_pool(name="psB", bufs=2, space="PSUM") as psB,
        tc.tile_pool(name="psF", bufs=2, space="PSUM") as psF,
    ):
        # ---- persistent SBUF tensors ----
        x_sb = big.tile([128, KT, S], BF16)          # xT: [p, ktile, s]
        wq_sb = big.tile([128, KT, 128], BF16)
        wk_sb = big.tile([128, KT, 128], BF16)
        wkq2_sb = big.tile([128, KT, 128], BF16)     # [k_h2 | q_h2]
        wv_sb = big.tile([128, KT, DH3], BF16)
        woA_sb = big.tile([128, D], BF16)            # Wo rows 0..127
        woB_sb = big.tile([64, D], BF16)             # Wo rows 128..191
        bias_sb = big.tile([128, 4], F32)
        ident_sb = big.tile([128, 128], BF16)
        qTA = big.tile([128, S], F32R)               # q^T heads 0,1
        kTA = big.tile([128, S], F32R)
        qTB = big.tile([128, S], F32R)               # head 2 in rows 64:128
        kTB = big.tile([128, S], F32R)
        v_sb = big.tile([128, ST, HPC, DH + 1], BF16)  # v rows + ones col
        ctx_sb = big.tile([128, ST, DH3], BF16)      # [q-part, qt, h*64+d]
        ctxTA = big.tile([128, S], BF16)             # ctx^T heads 0,1
        ctxTB = big.tile([64, S], BF16)              # ctx^T head 2

        # ---- DMA loads ----
        # The DMA bus is effectively serial (~360GB/s) and round-robins
        # across the SP/Pool/ACT queues, so the queue assignment below sets
        # the bus order: wq, wk, bias | x0, x1, wv | x2, x3, wkq2 | wo, ident.
        # single sync queue so the serial DMA bus moves bytes in exactly
        # the order the pipeline consumes them; bias rides the scalar queue
        # (tiny, interleaves once per round-robin round).
        nc.sync.dma_start(out=wq_sb, in_=wq.rearrange("p (kt m) -> p kt m", kt=KT))
        nc.scalar.dma_start(out=bias_sb, in_=bias[:, :])
        for c in range(4):
            cs = slice(c * 256, (c + 1) * 256)
            nc.sync.dma_start(
                out=x_sb[:, :, cs],
                in_=xT[:, cs].rearrange("(kt p) q -> p kt q", p=128),
            )
        nc.sync.dma_start(out=wk_sb, in_=wk.rearrange("p (kt m) -> p kt m", kt=KT))
        nc.sync.dma_start(out=wv_sb, in_=wv.rearrange("p (kt m) -> p kt m", kt=KT))
        nc.sync.dma_start(out=wkq2_sb, in_=wkq2.rearrange("p (kt m) -> p kt m", kt=KT))
        for c in range(2, 4):
            cs = slice(c * 512, (c + 1) * 512)
            nc.sync.dma_start(
                out=x_sb[:, :, cs],
                in_=xT[:, cs].rearrange("(kt p) q -> p kt q", p=128),
            )
        nc.sync.dma_start(out=woA_sb, in_=wo[0:128, :])
        nc.sync.dma_start(out=woB_sb, in_=wo[128:DH3, :])
        nc.sync.dma_start(out=ident_sb, in_=ident[:, :])
        nc.vector.memset(v_sb[:, :, :, DH : DH + 1], 1.0)

        # preload the Exp activation table during the DMA wait (the implicit
        # LoadActFuncSet lands before this dummy, off the critical path)
        dum_in = big.tile([1, 1], F32)
        dum_out = big.tile([1, 1], F32)
        nc.vector.memset(dum_in, 0.0)
        nc.scalar.activation(dum_out, dum_in, mybir.ActivationFunctionType.Exp)

        # warm the PE p-state during the x DMA wait: the Tensor engine ramps
        # 0.65 -> 1.2 -> 2.4 GHz only after ~3us of continuous execution and
        # the ramp resets when PE idles, so burn the DMA wait on throwaway
        # matmuls sized to end right as the first x chunk lands (~6us).
        warm = big.tile([128, 512], BF16)
        nc.vector.memset(warm, 0.0)
        for _ in range(8):
            pw = psS.tile([128, 512], F32, tag="s", name="ps_warm")
            nc.tensor.matmul(pw, lhsT=warm[:, 0:128], rhs=warm, start=True,
                             stop=True)

        # ---- projection helpers ----
        def proj_qk(pool, w_sb, cs, evict):
            n = cs.stop - cs.start
            ps = pool.tile([128, n], F32, tag=("s" if pool is psS else "f"),
                           name="ps_qk", padded_shape=None)
            for kt in range(KT):
                nc.tensor.matmul(
                    ps,
                    lhsT=w_sb[:, kt, :],
                    rhs=x_sb[:, kt, cs],
                    start=(kt == 0),
                    stop=(kt == KT - 1),
                )
            evict(ps, cs)

        def ev_k(ps, cs):
            nc.vector.tensor_scalar_add(kTA[:, cs], ps, bias_sb[:, 0:1])

        def ev_q(ps, cs):
            nc.vector.tensor_scalar_add(qTA[:, cs], ps, bias_sb[:, 2:3])


        def ev_kq2(ps, cs):
            # psum rows 0:64 = k_h2 (up-shift to 64:128), rows 64:128 = q_h2
            nc.vector.tensor_scalar_add(kTB[64:128, cs], ps[0:64, :], bias_sb[0:64, 1:2])
            nc.vector.tensor_scalar_add(qTB[64:128, cs], ps[64:128, :], bias_sb[64:128, 3:4])

        def proj_v(st, h):
            # one head's 64-col slice per call: spreads the v work so each
            # attention unit only pays for the head it consumes
            ss = slice(st * 128, (st + 1) * 128)
            ps_v = psF.tile([128, DH], F32, tag="f", name="ps_v")
            for kt in range(KT):
                nc.tensor.matmul(
                    ps_v,
                    lhsT=x_sb[:, kt, ss],
                    rhs=wv_sb[:, kt, h * DH : (h + 1) * DH],
                    start=(kt == 0),
                    stop=(kt == KT - 1),
                )
            nc.vector.tensor_copy(v_sb[:, st, h, 0:DH], ps_v)

        # head h (q/k)^T slices: heads 0,1 in kTA/qTA rows 0:64 / 64:128,
        # head 2 in kTB/qTB rows 64:128.
        def kq_rows(h):
            if h == 0:
                return kTA, qTA, slice(0, 64)
            if h == 1:
                return kTA, qTA, slice(64, 128)
            return kTB, qTB, slice(64, 128)

        # ---- attention pipeline pieces ----
        # stream of (h, q0, qw, j): per unit, j walks 16 key tiles over the
        # q-window [q0, q0+qw). g1 runs h2 first so the head-2 (ctxTB)
        # transpose block is ready early; the LAST unit is split into two
        # 512-wide subunits so the first half of its normalize/transpose/
        # out-projection overlaps the second half's exp stream (shorter
        # serial tail, at the cost of 16 narrower exps).
        units = [(0, 0, GW), (1, 0, GW), (2, 0, GW), (2, GW, GW),
                 (0, GW, GW), (1, GW, GW)]
        seq = [(h, q0, qw, j) for (h, q0, qw) in units for j in range(ST)]

        sc_tiles = {}   # (h, g, j) -> scores psum tile
        ctx_ps = {}     # (h, g, half) -> ctx psum tile

        def sc_step(h, q0, qw, j):
            # 512-wide matmuls: a matmul output must stay within one 2KB
            # PSUM bank (N <= 512 fp32)
            kk, qq, rows = kq_rows(h)
            ps = psS.tile([128, qw], F32, tag="s", name="ps_sc")
            for hs in range(qw // 512):
                nc.tensor.matmul(
                    ps[:, hs * 512 : (hs + 1) * 512],
                    lhsT=kk[rows, j * 128 : (j + 1) * 128],
                    rhs=qq[rows, q0 + hs * 512 : q0 + (hs + 1) * 512],
                    start=True,
                    stop=True,
                )
            sc_tiles[(h, q0, j)] = ps

        def exp_step(h, q0, qw, j, split=False, eng="act"):
            ps = sc_tiles.pop((h, q0, j))
            et = expp.tile([128, qw], BF16, tag="e", name="expT")
            if split:
                nc.scalar.activation(et[:, 0:512], ps[:, 0:512],
                                     mybir.ActivationFunctionType.Exp)
                nc.scalar.activation(et[:, 512:qw], ps[:, 512:qw],
                                     mybir.ActivationFunctionType.Exp)
            elif eng == "dve":
                # approximate exp on DVE (bit-trick), freeing ACT: the int16
                # write IS the bf16 exp tile by bitcast.
                nc.vector.tensor_scalar(
                    out=et.bitcast(I16), in0=ps,
                    scalar1=SCH_C1, scalar2=SCH_C2,
                    op0=mybir.AluOpType.mult, op1=mybir.AluOpType.add,
                )
            else:
                nc.scalar.activation(et, ps, mybir.ActivationFunctionType.Exp)
            return et

        def ctx_step(h, q0, qw, j, et):
            for half in range(qw // 512):
                key = (h, q0, half)
                if key not in ctx_ps:
                    ctx_ps[key] = psB.tile([128, 4 * (DH + 1)], F32, tag="b",
                                           name="ps_ctx")
                pc = ctx_ps[key]
                for qq in range(4):
                    # start marks the WHOLE 2KB psum bank pending-zero, so
                    # only the first write of the bank's group may set it
                    # (qq>0 first-writes land on pending bytes = overwrite).
                    qloc = half * 4 + qq
                    nc.tensor.matmul(
                        pc[:, qq * (DH + 1) : (qq + 1) * (DH + 1)],
                        lhsT=et[:, qloc * 128 : (qloc + 1) * 128],
                        rhs=v_sb[:, j, h, :],
                        start=(j == 0 and qq == 0),
                        stop=(j == ST - 1 and qq == 3),
                        skip_group_check=True,
                    )

        def norm_evict(h, q0, half, last=False):
            # psum [128, 4*(65)]: per qq, cols 0:64 = ctx, col 64 = denom.
            pc = ctx_ps.pop((h, q0, half))
            v3 = pc.rearrange("p (qq c) -> p qq c", c=DH + 1)
            den = work.tile([128, 4], F32, tag="den", name="den")
            nc.vector.tensor_copy(den, v3[:, :, DH : DH + 1].squeeze(-1))
            rcp = work.tile([128, 4], F32, tag="rcp", name="rcp")
            nc.vector.reciprocal_approx_fast(out=rcp, in_=den)
            qt0 = q0 // 128 + half * 4
            if last:
                # final unit: ACT is idle after the last exp — normalize
                # there (Copy with per-partition scale), one qq per instr,
                # in parallel with DVE doing the other half
                for qq in range(4):
                    nc.scalar.activation(
                        ctx_sb[:, qt0 + qq, h * DH : (h + 1) * DH],
                        v3[:, qq, 0:DH],
                        mybir.ActivationFunctionType.Copy,
                        scale=rcp[:, qq : qq + 1],
                    )
                return
            nc.vector.tensor_mul(
                ctx_sb[:, qt0 : qt0 + 4, h * DH : (h + 1) * DH],
                v3[:, :, 0:DH],
                rcp.unsqueeze(-1).broadcast_to([128, 4, DH]),
            )

        # ---- transpose + out-projection ----
        def trans_block(g, h, half):
            # ctx [q, d] -> ctx^T [d, q] for one head, 4 q-subtiles, via PE
            # transpose. Per-head so each block is ready right after that
            # head's norm_evict: h0 -> ctxTA rows 0:64, h1 -> ctxTA rows
            # 64:128, h2 -> ctxTB rows 0:64.
            rows = slice(64, 128) if h == 1 else slice(0, 64)
            dst = ctxTB if h == 2 else ctxTA
            pt = psF.tile([128, 512], BF16, tag="f", name="ps_t")
            for qq in range(4):
                qt = g * 8 + half * 4 + qq
                nc.tensor.transpose(
                    pt[rows, qq * 128 : (qq + 1) * 128],
                    ctx_sb[:, qt, h * DH : (h + 1) * DH], ident_sb,
                )
            cs = slice(g * GW + half * 512, g * GW + (half + 1) * 512)
            nc.vector.tensor_copy(dst[rows if h == 1 else slice(0, 64), cs],
                                  pt[rows, :])

        o_tiles = {}

        def outproj_chunk(qt, c, pool, tag):
            # c=0: cols 0:512, c=1: cols 512:768 (via psF filler pool);
            # pool=psS at the tail does the full row in one [128, 768] tile.
            if pool is psS:
                osl = slice(0, D)
            else:
                osl = slice(c * 512, 512 if c == 0 else D)
            n = osl.stop - osl.start
            po = pool.tile([128, n], F32, tag=tag, name="ps_o")
            nc.tensor.matmul(
                po, lhsT=ctxTA[:, qt * 128 : (qt + 1) * 128],
                rhs=woA_sb[:, osl], start=True, stop=False,
            )
            nc.tensor.matmul(
                po, lhsT=ctxTB[:, qt * 128 : (qt + 1) * 128],
                rhs=woB_sb[:, osl], start=False, stop=True,
            )
            ss = slice(qt * 128, (qt + 1) * 128)
            if qt not in o_tiles:
                o_tiles[qt] = outp.tile([128, D], BF16, tag="o", name="o_sb")
            o_sb = o_tiles[qt]
            if pool is psS and tag == "act":
                # tail odd tiles: evict on the (post-exp idle) ACT engine
                nc.scalar.activation(o_sb[:, osl], po,
                                     mybir.ActivationFunctionType.Copy)
            else:
                nc.vector.tensor_copy(o_sb[:, osl], po)
            if osl.stop == D:
                nc.sync.dma_start(out=out[ss, :], in_=o_sb)

        def outproj_tail(qt, kind):
            # kind 0: one [128, 768] psS tile, DVE evict, sync DMA.
            # kind 1: two psF chunks, ACT Copy evicts, Pool-queue DMA.
            # Alternating kinds gives 4 psum tiles and 2 evict engines in
            # flight, so the tail streams at matmul rate.
            ss = slice(qt * 128, (qt + 1) * 128)
            o_sb = outp.tile([128, D], BF16, tag="o", name="o_sb")
            if kind == 0:
                po = psS.tile([128, D], F32, tag="s", name="ps_o")
            for osl in (slice(0, 512), slice(512, D)):
                if kind == 0:
                    pr = po[:, osl]
                else:
                    pr = psF.tile([128, osl.stop - osl.start], F32, tag="f",
                                  name="ps_o")
                # ctxTB part first: it is ready well before the tail, so PE
                # streams it while the h1 transpose eviction still lands
                nc.tensor.matmul(
                    pr, lhsT=ctxTB[:, qt * 128 : (qt + 1) * 128],
                    rhs=woB_sb[:, osl], start=True, stop=False,
                )
                nc.tensor.matmul(
                    pr, lhsT=ctxTA[:, qt * 128 : (qt + 1) * 128],
                    rhs=woA_sb[:, osl], start=False, stop=True,
                )
                if kind == 1:
                    nc.scalar.activation(o_sb[:, osl], pr,
                                         mybir.ActivationFunctionType.Copy)
            if kind == 0:
                nc.vector.tensor_copy(o_sb, po)
            nc.sync.dma_start(out=out[ss, :], in_=o_sb)

        def outproj_last(qt):
            # final tile: halves evicted concurrently on DVE and ACT into
            # separate tiles, each DMA'd immediately — shortens the final
            # evict->DMA->sem chain that nothing can overlap.
            ss = slice(qt * 128, (qt + 1) * 128)
            po = psS.tile([128, D], F32, tag="s", name="ps_o")
            for osl in (slice(0, 512), slice(512, D)):
                nc.tensor.matmul(
                    po[:, osl], lhsT=ctxTA[:, qt * 128 : (qt + 1) * 128],
                    rhs=woA_sb[:, osl], start=True, stop=False,
                )
                nc.tensor.matmul(
                    po[:, osl], lhsT=ctxTB[:, qt * 128 : (qt + 1) * 128],
                    rhs=woB_sb[:, osl], start=False, stop=True,
                )
            oa = outp.tile([128, 384], BF16, tag="oa", name="oa_sb")
            ob = outp.tile([128, 384], BF16, tag="ob", name="ob_sb")
            nc.vector.tensor_copy(oa, po[:, 0:384])
            nc.scalar.activation(ob, po[:, 384:D],
                                 mybir.ActivationFunctionType.Copy)
            nc.sync.dma_start(out=out[ss, 0:384], in_=oa)
            nc.sync.dma_start(out=out[ss, 384:D], in_=ob)

        # ---- startup: projections needed before the exp stream starts ----
        # q g0 in 256-col pieces pipelined behind the x DMA pieces; k cols
        # 0:384 cover sc j=0..2 (the rest stream in as fillers). sc(0)/sc(1)
        # emitted as early as their operands allow — the v tiles (only
        # needed by ctx) come after.
        for c in range(4):
            proj_qk(psS, wq_sb, slice(c * 256, (c + 1) * 256), ev_q)
        proj_qk(psS, wk_sb, slice(0, 128), ev_k)

        # ---- per-step filler schedule ----
        # sched[i] = closures emitted at pipeline step i. EMISSION-ORDER
        # LAW: a consumer emitted before its producer gets NO dependency
        # edge and reads garbage, so placement rules are strict:
        #   k chunk col j*128     -> before step j-2 (sc(j) emission)
        #   v piece (st, h)       -> before the step of ctx(h-unit, j=st)
        #   kq2 chunk c           -> before step min(28+c, 30) (qTB whole-g0
        #                            read by sc((2,0),0) at step 30)
        #   q-g1 chunk            -> before step 62 (unit (0,1) sc's)
        #   g0 transposes         -> after step 47 (all g0 norms emitted)
        #   TB-g1 after 63, TA-h0-g1 after 79 (their norms)
        def mk_qk(w_sb, cs, evict):
            return lambda: proj_qk(psF, w_sb, cs, evict)

        k2cs = [slice(128 * i, 128 * (i + 1)) for i in range(16)]
        qcs = [slice(1024 + 128 * i, 1024 + 128 * (i + 1)) for i in range(8)]
        sched = {}

        def put(slot, f):
            sched.setdefault(slot, []).append(f)

        def mk_v(st, h):
            return lambda: proj_v(st, h)

        for i in range(12):  # k chunks for sc j=4..15
            put(i, mk_qk(wk_sb, slice((i + 4) * 128, (i + 5) * 128), ev_k))
        for st in range(2, 16):  # v h0 pieces for unit (0,0)
            put(min(st - 2, 13), mk_v(st, 0))
        put(14, mk_v(0, 1))
        put(14, mk_v(1, 1))
        for st in range(2, 8):  # v h1 pieces for unit (1,0)
            put(14 + st, mk_v(st, 1))
        for c in range(8):  # kq2 cols 0:1024 (all read at step 30)
            put(22 + c, mk_qk(wkq2_sb, k2cs[c], ev_kq2))
        for st in range(8, 16):  # v h1 pieces after the kq2 chunk per slot
            put(14 + st, mk_v(st, 1))
        for c in range(8, 16):  # kq2 cols 1024:2048 before the v pieces
            put(22 + c, mk_qk(wkq2_sb, k2cs[c], ev_kq2))
        for st in range(8):
            put(30 + st, mk_v(st, 2))
        for st in range(8, 16):
            put(30 + st, mk_v(st, 2))
        put(44, mk_qk(wq_sb, qcs[0], ev_q))
        put(45, mk_qk(wq_sb, qcs[1], ev_q))
        for i in range(2, 8):
            put(46 + i, mk_qk(wq_sb, qcs[i], ev_q))
        slot = 55
        for h in range(HPC):
            for half in range(2):
                put(slot, lambda h=h, half=half: trans_block(0, h, half))
                slot += 1
        opj_slots = [61] + list(range(64, 79))
        k = 0
        for qt in range(8):  # g0 out-projection, two psF chunks each
            for c in range(2):
                put(opj_slots[k], lambda qt=qt, c=c: outproj_chunk(qt, c, psF, "f"))
                k += 1
        for half in range(2):
            put(64 + half, lambda half=half: trans_block(1, 2, half))
        for half in range(2):
            put(80 + half, lambda half=half: trans_block(1, 0, half))

        # ---- main pipelined emission ----
        sc_step(*seq[0])
        proj_qk(psF, wk_sb, slice(128, 256), ev_k)
        sc_step(*seq[1])
        proj_qk(psF, wk_sb, slice(256, 384), ev_k)
        proj_v(0, 0)
        proj_v(1, 0)
        proj_qk(psF, wk_sb, slice(384, 512), ev_k)
        for i, (h, q0, qw, j) in enumerate(seq):
            eng = "dve" if (i % 4 == 3 and i < 80) else "act"
            et = exp_step(h, q0, qw, j, eng=eng)
            ctx_step(h, q0, qw, j, et)
            if i + 2 < len(seq):
                sc_step(*seq[i + 2])
            if j == ST - 1:
                for half in range(qw // 512):
                    norm_evict(h, q0, half)
            for f in sched.pop(i, []):
                f()
        assert not sched, f"unemitted filler slots: {sorted(sched)}"

        # ---- tail: transpose the h1 block of g1 + out-proj qt 8..15 ----
        trans_block(1, 1, 0)
        trans_block(1, 1, 1)
        for qt in range(8, 16):
            outproj_tail(qt, qt % 2)

        if debug:
            nc.sync.dma_start(out=d_qTA[:, :], in_=qTA.bitcast(F32))
            nc.sync.dma_start(out=d_kTA[:, :], in_=kTA.bitcast(F32))
            nc.sync.dma_start(out=d_qTB[:, :], in_=qTB.bitcast(F32))
            nc.sync.dma_start(out=d_kTB[:, :], in_=kTB.bitcast(F32))
            vf = work.tile([128, ST * HPC * (DH + 1)], F32, tag="dv", name="vf")
            nc.vector.tensor_copy(vf, v_sb.rearrange("p a b c -> p (a b c)"))
            nc.sync.dma_start(out=d_v[:, :], in_=vf)
            cf = work.tile([128, ST * DH3], F32, tag="dc", name="cf")
            nc.vector.tensor_copy(cf, ctx_sb.rearrange("p a b -> p (a b)"))
            nc.sync.dma_start(out=d_ctx[:, :], in_=cf)
            caf = work.tile([128, S], F32, tag="dca", name="caf")
            nc.vector.tensor_copy(caf, ctxTA)
            nc.sync.dma_start(out=d_ctxTA[:, :], in_=caf)
            cbf = work.tile([64, S], F32, tag="dcb", name="cbf")
            nc.vector.tensor_copy(cbf, ctxTB)
            nc.sync.dma_start(out=d_ctxTB[:, :], in_=cbf)

    nc.compile()
    return nc


def _w_rearrange(w):
    """[768, M] -> [128, 6*M] bf16: row p holds w[kt*128+p, :] for kt=0..5,
    so the device DMA is one contiguous segment per partition."""
    import ml_dtypes

    d, m = w.shape
    kt = d // 128
    return np.ascontiguousarray(
        w.reshape(kt, 128, m).transpose(1, 0, 2).reshape(128, kt * m)
    ).astype(ml_dtypes.bfloat16)


def _bias_block(bq, bk, col):
    # [128, 4]: col0 = bk heads01, col1 = bk head2 (rows 0:64),
    # col2 = bq heads01 (pre-scaled), col3 = bq head2 at rows 64:128
    blk = np.zeros((128, 4), np.float32)
    blk[:, 0] = bk[col : col + 128]
    blk[0:64, 1] = bk[col + 128 : col + 192]
    blk[:, 2] = bq[col : col + 128] * np.float32(0.125)
    blk[64:128, 3] = bq[col + 128 : col + 192] * np.float32(0.125)
    return blk


def _prep_in_maps(inputs):
    import ml_dtypes

    bf16 = ml_dtypes.bfloat16
    x = np.asarray(inputs["x"], dtype=np.float32)
    Wq = np.asarray(inputs["Wq"], dtype=np.float32)
    Wk = np.asarray(inputs["Wk"], dtype=np.float32)
    Wv = np.asarray(inputs["Wv"], dtype=np.float32)
    Wo = np.asarray(inputs["Wo"], dtype=np.float32)
    bq = np.asarray(inputs["bq"], dtype=np.float32)
    bk = np.asarray(inputs["bk"], dtype=np.float32)
    ident = np.eye(128, dtype=np.float32).astype(bf16)

    in_maps = []
    for c in range(NCORES):
        b = c // 4
        col = (c % 4) * DH3
        sl = slice(col, col + DH3)
        in_maps.append(
            {
                "xT": np.ascontiguousarray(x[b].T).astype(bf16),
                "wq": _w_rearrange(Wq[:, col : col + 128] * np.float32(0.125)),
                "wk": _w_rearrange(Wk[:, col : col + 128]),
                "wkq2": _w_rearrange(np.concatenate(
                    [
                        Wk[:, col + 128 : col + 192],
                        Wq[:, col + 128 : col + 192] * np.float32(0.125),
                    ],
                    axis=1,
                )),
                "wv": _w_rearrange(Wv[:, sl]),
                "wo": np.ascontiguousarray(Wo[sl, :]).astype(bf16),
                "bias": _bias_block(bq, bk, col),
                "ident": ident,
            }
        )
    return in_maps


def _combine(results, inputs):
    Wo = np.asarray(inputs["Wo"], dtype=np.float32)
    bv = np.asarray(inputs["bv"], dtype=np.float32)
    bo = np.asarray(inputs["bo"], dtype=np.float32)
    base = bv @ Wo + bo  # [D]
    out = np.empty((B, S, D), dtype=np.float32)
    for b in range(B):
        acc = results[4 * b]["out"].astype(np.float32)
        for c in range(4 * b + 1, 4 * b + 4):
            acc = acc + results[c]["out"].astype(np.float32)
        out[b] = acc + base
    return out


def run(inputs, trace: bool = False):
    """Run the 8-core kernel; returns (output, BassKernelResults)."""
    global _CACHED_NC
    if _CACHED_NC is None:
        _CACHED_NC = _build_nc()
    in_maps = _prep_in_maps(inputs)
    try:
        res = run_bass_kernel_spmd(
            _CACHED_NC, in_maps, core_ids=list(range(NCORES)), trace=trace
        )
    except ModuleNotFoundError:
        import os

        os.environ["BASS_NEVER_TRACE"] = "1"
        res = run_bass_kernel_spmd(
            _CACHED_NC, in_maps, core_ids=list(range(NCORES)), trace=False
        )
    return _combine(res.results, inputs), res


def kernel(**inputs) -> np.ndarray:
    out, _ = run(inputs)
    return out

